# revision 62
# baseline (speedup 1.0000x reference)
"""Trainium2 Bass kernel for the GAT block (masked attention + SwiGLU MLP).

Sharding: token-split across 8 cores. Core c handles batch b = c//4 and the
512-query slice starting at (c%4)*512 of that batch. Each core computes
full-batch K/V projections (duplicated across the 4 cores of a batch -- no
collectives), its own queries' attention, and the MLP for its token slice.

Device-side strategy (v3, tuned against the TimelineSim cost model):
  - weights, activations-in-flight and x are bf16 (matmul moving operands at
    1 cycle/row on PE; f32 would be 4x); residual h and PSUM stay f32.
  - rmsnorm: x^2 row-sums (DVE STT with accum), batched sqrt (Act) +
    reciprocal (DVE), scale to bf16 z (Act/DVE/Pool rotation), PE transpose
    via bf16 identity; evacuation alternates DVE/Act.
  - scores computed transposed (sT[keys, queries]) with two heads packed per
    128-row tile (tile_position); exp on Act straight from PSUM (the 64
    [128,1024] exps, ~64us, pace the whole attention phase); mask multiply on
    DVE (bf16 2x mode); softmax denominators ride as a 65th row via a ones
    column in V; bv folded into the xb residual input.
  - schedule: attention round pr computes kq(pr)+scores(pr) while weaving in
    the AV matmuls of round pr-1 (whose p tiles are ready), so PE never
    stalls on the exp stream; round 3 runs av(2) at double rate and trails
    av(3) by two groups in a spare PSUM pair.
  - DMAs are batched (one per weight matrix / bias pack / mask, ~36 total vs
    116; each DMA costs ~625ns of exclusive HWDGE) and ordered x-first.
  - activation-table discipline: Sqrt/Exp/Silu live in different Act tables
    (1.3us reload each); all sqrts precede the exp phase, Silu only appears
    in the MLP phase, and warm-up activations preload tables off-path.
  - PSUM 8 banks: scores ring 2x[128,1024], kq/v tile [128,1024], AV/
    transpose pair 2x[128,512], MLP ring 2x[128,512] after attention frees.
"""
import os
import sys

sys.path.insert(0, "/opt/trn_rl_repo")

# CoreSim doesn't implement Silu; sim runs decompose it into Sigmoid+mul.
SIM_SILU = os.environ.get("KSIM_SILU") == "1"

from contextlib import ExitStack

import ml_dtypes
import numpy as np

import concourse.bass as bass
import concourse.mybir as mybir
import concourse.tile as tile
from concourse import bacc
from concourse.masks import make_identity

D = 512
N = 2048
B = 2
HEADS = 8
HD = 64
HDIM = 2048
NCORES = 8
QT = 512  # tokens (queries) per core
EPS = float(np.finfo(np.float32).eps)

F32 = mybir.dt.float32
F32R = mybir.dt.float32r
BF16 = mybir.dt.bfloat16

AF = mybir.ActivationFunctionType
ALU = mybir.AluOpType

DT4 = D // 128    # 4 feature tiles
TT = N // 128     # 16 token tiles (full batch)
QTT = QT // 128   # 4 own-query tiles
HT = HDIM // 128  # 16 hidden tiles
NCH = 2           # query chunks
CQ = QT // NCH    # 256 queries per chunk
CQT = CQ // 128   # 2 query tiles per chunk
V65 = HD + 1

# bias pack column offsets
BQ0, BK0, B10, B20, B30 = 0, 4, 8, 24, 40
NBIAS = 44


def build_module(reps=1):
    nc = bacc.Bacc(
        "TRN2", target_bir_lowering=False, debug=False, num_devices=NCORES)

    p = {}
    def param(name, shape, dtype=F32, out=False):
        p[name] = nc.declare_dram_parameter(name, shape, dtype, isOutput=out)
        return p[name]

    param("xf", [N, D], BF16)      # full batch x
    param("xo", [QT, D], BF16)     # own-slice x (norm only)
    param("xb", [QT, D], BF16)     # own-slice x + bv (residual base)
    param("mT", [N, QT], BF16)     # mask transposed [keys, queries], 0/1
    param("wqT", [D, D], BF16)     # (Wq*g1).T / 8
    param("wkT", [D, D], BF16)     # (Wk*g1).T
    param("wvT", [D, D], BF16)     # (Wv*g1).T
    param("w1T", [D, HDIM], BF16)  # (W1*g2).T
    param("w2T", [D, HDIM], BF16)  # (W2*g2).T
    param("w3T", [HDIM, D], BF16)  # W3.T
    param("bias", [128, NBIAS])    # packed bq8|bk|b1|b2|b3
    param("out", [QT, D], BF16, out=True)

    with ExitStack() as ctx:
        tc = ctx.enter_context(tile.TileContext(nc))
        for _ in range(reps):
            with ExitStack() as rctx:
                _body(rctx, tc, nc, p)
    nc.compile()
    return nc


def _body(ctx, tc, nc, p):
    # ---------- long-lived pools ----------
    persist = ctx.enter_context(tc.tile_pool(name="persist", bufs=1))
    small = ctx.enter_context(tc.tile_pool(name="small", bufs=8))
    rot = ctx.enter_context(tc.tile_pool(name="rot", bufs=3))
    azone = ctx.enter_context(tc.tile_pool(name="azone", bufs=1, side="right"))

    identf = persist.tile([128, 128], F32, tag="identf", name="identf")
    make_identity(nc, identf[:])
    identb = persist.tile([128, 128], BF16, tag="identb", name="identb")
    nc.gpsimd.tensor_copy(identb[:], identf[:])
    epsb = persist.tile([128, 1], F32, tag="epsb", name="epsb")
    nc.gpsimd.memset(epsb[:], EPS)
    warm = persist.tile([128, 1], F32, tag="warm", name="warm")
    nc.scalar.activation(warm[:], epsb[:], AF.Sqrt)

    xb_s = [persist.tile([128, D], BF16, tag=f"xb{q}", name=f"xb{q}")
            for q in range(QTT)]
    hbuf = [persist.tile([128, D], F32, tag=f"hb{q}", name=f"hb{q}")
            for q in range(QTT)]
    outbuf = [persist.tile([128, D], BF16, tag=f"ob{q}", name=f"ob{q}")
              for q in range(QTT)]
    bias_t = persist.tile([128, NBIAS], F32, tag="bias", name="bias")

    def bias_ap(base, i):
        return bias_t[:, base + i:base + i + 1]

    # mask, resident for the whole attention phase
    mT_t = azone.tile([128, TT * QT], BF16, tag="mT", name="mT")
    mTv = mT_t[:].rearrange("p (t q) -> p t q", t=TT)

    # z + qkv weights scope (closes after attention chunk A)
    zpool = ExitStack()
    zp = zpool.enter_context(tc.tile_pool(name="zp", bufs=1))
    wqkv = zpool.enter_context(tc.tile_pool(name="wqkv", bufs=1))

    # ---------- front scope: x tiles + norm-transpose ----------
    s_front = ExitStack()
    xpool = s_front.enter_context(tc.tile_pool(name="xpool", bufs=1))
    fscr = s_front.enter_context(tc.tile_pool(name="fscr", bufs=2))
    ftr_ps = s_front.enter_context(
        tc.tile_pool(name="ftr_ps", bufs=2, space="PSUM"))

    xf_s = [xpool.tile([128, D], BF16, tag=f"xf{t}", name=f"xf{t}")
            for t in range(TT)]
    xo_s = [xpool.tile([128, D], BF16, tag=f"xq{q}", name=f"xq{q}")
            for q in range(QTT)]

    # ---- DMA issue order: all of x first (batched), then weights ----
    for t in range(TT):
        nc.sync.dma_start(xf_s[t][:], p["xf"][t * 128:(t + 1) * 128, :])
    for q in range(QTT):
        nc.sync.dma_start(xo_s[q][:], p["xo"][q * 128:(q + 1) * 128, :])
    for q in range(QTT):
        nc.sync.dma_start(xb_s[q][:], p["xb"][q * 128:(q + 1) * 128, :])

    wk_t = wqkv.tile([128, DT4 * D], BF16, tag="wk", name="wk")
    wq_t = wqkv.tile([128, DT4 * D], BF16, tag="wq", name="wq")
    wv_t = wqkv.tile([128, DT4 * D], BF16, tag="wv", name="wv")

    def wslice(w, dk, lo, hi):
        return w[:, dk * D + lo:dk * D + hi]

    nc.sync.dma_start(
        wk_t[:].rearrange("p (a d) -> p a d", a=DT4),
        p["wkT"][:].rearrange("(a p) d -> p a d", p=128))
    nc.sync.dma_start(
        wq_t[:].rearrange("p (a d) -> p a d", a=DT4),
        p["wqT"][:].rearrange("(a p) d -> p a d", p=128))
    nc.sync.dma_start(
        wv_t[:].rearrange("p (a d) -> p a d", a=DT4),
        p["wvT"][:].rearrange("(a p) d -> p a d", p=128))
    nc.sync.dma_start(bias_t[:], p["bias"][:])
    nc.sync.dma_start(
        mT_t[:].rearrange("p (t q) -> p t q", t=TT),
        p["mT"][:].rearrange("(t p) q -> p t q", p=128))

    # normalized transposed activations
    zT_all = zp.tile([128, DT4 * N], BF16, tag="zT", name="zT")
    zoT_all = zp.tile([128, DT4 * QT], BF16, tag="zoT", name="zoT")
    zT = [zT_all[:, d * N:(d + 1) * N] for d in range(DT4)]
    zoT = [zoT_all[:, d * QT:(d + 1) * QT] for d in range(DT4)]

    def norm_transpose_tile(xt, inv_ap, dst_all, ncols, col0, eng_i):
        """raw token-major bf16 tile -> normalized feature-major bf16
        columns. Scale engine rotates Act/DVE/Pool; transpose is bf16."""
        zt = rot.tile([128, D], BF16, tag="zt", name="zt")
        if eng_i % 3 == 0:
            nc.scalar.mul(zt[:], xt[:], inv_ap)
        elif eng_i % 3 == 1:
            nc.vector.tensor_scalar_mul(zt[:], xt[:], inv_ap)
        else:
            nc.gpsimd.tensor_scalar_mul(zt[:], xt[:], inv_ap)
        ps = ftr_ps.tile([128, D], BF16, tag="ftr", name="ftr")
        for d in range(DT4):
            nc.tensor.matmul(ps[:, d * 128:(d + 1) * 128],
                             zt[:, d * 128:(d + 1) * 128],
                             identb[:], is_transpose=True,
                             start=(d == 0), stop=(d == DT4 - 1))
        dst = dst_all[:].rearrange("p (d c) -> p d c", c=ncols)[
            :, :, col0:col0 + 128]
        src = ps[:].rearrange("p (d c) -> p d c", c=128)
        if eng_i % 2 == 0:
            nc.vector.tensor_copy(dst, src)
        else:
            nc.scalar.copy(dst, src)

    def front_group(tiles, dsts):
        """tiles: list of (xt, dst_all, ncols, col0). Batched sqrt/recip."""
        G = len(tiles)
        sss = small.tile([128, G], F32, tag="sss", name="sss")
        srtg = small.tile([128, G], F32, tag="srtg", name="srtg")
        invg = small.tile([128, G], F32, tag="invg", name="invg")
        for i, (xt, _, _, _) in enumerate(tiles):
            scr = fscr.tile([128, D], BF16, tag=f"scr{i % 2}", name="scr")
            nc.vector.scalar_tensor_tensor(
                out=scr[:], in0=xt[:], scalar=1.0, in1=xt[:],
                op0=ALU.mult, op1=ALU.mult, accum_out=sss[:, i:i + 1])
        nc.scalar.activation(srtg[:], sss[:], AF.Sqrt, bias=epsb[:],
                             scale=1.0 / D)
        nc.vector.reciprocal(invg[:], srtg[:])
        for i, (xt, dst_all, ncols, col0) in enumerate(tiles):
            norm_transpose_tile(xt, invg[:, i:i + 1], dst_all, ncols, col0,
                                dsts[0] + i)
        dsts[0] += G

    eng_ctr = [0]
    splits = [(0, 2), (2, 4), (4, 8), (8, 12), (12, 16)]
    for lo, hi in splits:
        front_group([(xf_s[t], zT_all, N, t * 128) for t in range(lo, hi)],
                    eng_ctr)
    front_group([(xo_s[q], zoT_all, QT, q * 128) for q in range(QTT)],
                eng_ctr)

    s_front.close()  # frees x tiles, front scratch + psum

    # ---- W1/W2: issue loads now (transfers overlap attention) ----
    s_mlpw = ExitStack()
    mwp = s_mlpw.enter_context(
        tc.tile_pool(name="mwp", bufs=1, side="right"))
    w1_t = mwp.tile([128, DT4 * HDIM], BF16, tag="w1", name="w1")
    w2_t = mwp.tile([128, DT4 * HDIM], BF16, tag="w2", name="w2")
    nc.sync.dma_start(
        w1_t[:].rearrange("p (a h) -> p a h", a=DT4),
        p["w1T"][:].rearrange("(a p) h -> p a h", p=128))
    nc.sync.dma_start(
        w2_t[:].rearrange("p (a h) -> p a h", a=DT4),
        p["w2T"][:].rearrange("(a p) h -> p a h", p=128))

    def w1slice(w, dk, lo, hi):
        return w[:, dk * HDIM + lo:dk * HDIM + hi]

    # ---------- attention operands ----------
    kT = [azone.tile([128, N], BF16, tag=f"kT{pr}", name=f"kT{pr}")
          for pr in range(DT4)]
    qT = [azone.tile([128, QT], BF16, tag=f"qT{pr}", name=f"qT{pr}")
          for pr in range(DT4)]
    v65_all = azone.tile([128, TT * HEADS * V65], BF16, tag="v65", name="v65")
    v65 = [v65_all[:, t * HEADS * V65:(t + 1) * HEADS * V65]
           for t in range(TT)]
    # p tiles: one buffer per sub, reused across head pairs (scores(pr+1)
    # only starts after av(pr) has consumed the buffer)
    pt_pool = ExitStack()
    ptp = pt_pool.enter_context(tc.tile_pool(name="ptp", bufs=1, side="right"))
    p_t = [ptp.tile([128, TT * QT], BF16, tag=f"pt{sub}", name=f"pt{sub}")
           for sub in (0, 1)]

    # hn / MLP buffers + w3: allocated later, in the zone zT/wqkv vacate
    hnT = [None] * DT4
    gbuf = [None] * HT
    w3_holder = [None]

    def w3slice(j, lo, hi):
        return w3_holder[0][:, j * D + lo:j * D + hi]

    # ---------- PSUM rings ----------
    s_scA = ExitStack()
    sb_ps = ctx.enter_context(
        tc.tile_pool(name="sb_ps", bufs=1, space="PSUM", side="right"))
    sc_ps = s_scA.enter_context(
        tc.tile_pool(name="sc_ps", bufs=1, space="PSUM", side="right"))
    vq_ps = s_scA.enter_context(
        tc.tile_pool(name="vq_ps", bufs=1, space="PSUM", side="right"))

    sc_ring = [
        lambda: sc_ps.tile([128, 1024], F32, tag="sca", name="sca"),
        lambda: sc_ps.tile([128, 1024], F32, tag="scb", name="scb"),
    ]
    sc_state = [0]

    def sc_tile(ring=None):
        r = ring if ring is not None else sc_ring
        t = r[sc_state[0] % len(r)]()
        sc_state[0] += 1
        return t

    def vq_tile():
        return vq_ps.tile([128, 1024], F32, tag="vq", name="vq")

    sb_tiles = [
        lambda: sb_ps.tile([128, 512], F32, tag="sba", name="sba"),
        lambda: sb_ps.tile([128, 512], F32, tag="sbb", name="sbb"),
    ]
    sb_state = [0]

    def sb_tile():
        t = sb_tiles[sb_state[0] % 2]()
        sb_state[0] += 1
        return t

    # ---------- building blocks ----------
    def kq_block(pr):
        """project kT[pr] (full batch) and qT[pr] (own queries)."""
        for half in (0, 1):
            ps = vq_tile()
            for qtr in (0, 1):
                for dk in range(DT4):
                    nc.tensor.matmul(
                        ps[:, qtr * 512:(qtr + 1) * 512],
                        wslice(wk_t, dk, pr * 128, (pr + 1) * 128),
                        zT[dk][:, half * 1024 + qtr * 512:
                               half * 1024 + (qtr + 1) * 512],
                        start=(dk == 0), stop=(dk == DT4 - 1))
            nc.vector.tensor_scalar_add(
                kT[pr][:, half * 1024:(half + 1) * 1024], ps[:],
                bias_ap(BK0, pr))
        ps = sb_tile()
        for dk in range(DT4):
            nc.tensor.matmul(
                ps[:, 0:QT],
                wslice(wq_t, dk, pr * 128, (pr + 1) * 128),
                zoT[dk][:], start=(dk == 0), stop=(dk == DT4 - 1))
        nc.vector.tensor_scalar_add(qT[pr][:], ps[:, 0:QT], bias_ap(BQ0, pr))

    def v_block(g2):
        """project v for token tiles 2*g2, 2*g2+1 into v65 (token-major)."""
        ps = vq_tile()
        for tt in range(2):
            t = 2 * g2 + tt
            for dk in range(DT4):
                nc.tensor.matmul(
                    ps[:, tt * 512:(tt + 1) * 512],
                    zT[dk][:, t * 128:(t + 1) * 128],
                    wslice(wv_t, dk, 0, D),
                    start=(dk == 0), stop=(dk == DT4 - 1))
        dst = v65_all[:, g2 * 2 * HEADS * V65:(g2 + 1) * 2 * HEADS * V65]
        dv = dst.rearrange("q (t h c) -> q t h c", t=2, c=V65)
        sv = ps[:].rearrange("q (t h c) -> q t h c", t=2, c=HD)
        nc.scalar.copy(dv[:, 0:1, :, 0:HD], sv[:, 0:1])
        nc.vector.tensor_copy(dv[:, 1:2, :, 0:HD], sv[:, 1:2])

    def sc_g(pr, g, ring=None):
        """scores + exp + mask for key-tile pair g of head pair pr."""
        ps_pair = [sc_tile(ring) for _ in (0, 1)]
        for half in (0, 1):
            kt = 2 * g + half
            for sub in (0, 1):
                nc.tensor.matmul(
                    ps_pair[sub][:, half * 512:(half + 1) * 512],
                    kT[pr][64 * sub:64 * (sub + 1),
                           kt * 128:(kt + 1) * 128],
                    qT[pr][64 * sub:64 * (sub + 1), :],
                    start=True, stop=True,
                    tile_position=(64 * sub, 0))
        for sub in (0, 1):
            praw = rot.tile([128, 1024], BF16, tag="praw", name="praw", bufs=4)
            nc.scalar.activation(praw[:], ps_pair[sub][:], AF.Exp)
            nc.vector.tensor_mul(
                p_t[sub][:, g * 1024:(g + 1) * 1024].rearrange(
                    "p (t q) -> p t q", t=2),
                praw[:].rearrange("p (t q) -> p t q", t=2),
                mTv[:, 2 * g:2 * g + 2, :])

    av_ps = [None, None]

    def av_start():
        av_ps[0] = sb_tile()
        av_ps[1] = sb_tile()

    def av_chunk(pr, g):
        """two AV accumulation steps (key tiles 2g, 2g+1) for both heads."""
        for sub in (0, 1):
            h = 2 * pr + sub
            for half in (0, 1):
                kt = 2 * g + half
                nc.tensor.matmul(av_ps[sub][0:V65, 0:QT],
                                 v65[kt][:, V65 * h:V65 * (h + 1)],
                                 p_t[sub][:, kt * 512:(kt + 1) * 512],
                                 start=(kt == 0), stop=(kt == TT - 1))

    def av_epilogue(pr):
        for sub in (0, 1):
            h = 2 * pr + sub
            oT = rot.tile([V65, QT], F32, tag="oT", name="oT", bufs=2)
            nc.scalar.copy(oT[:], av_ps[sub][0:V65, 0:QT])
            for qc in range(QTT):
                ps_t = sb_ps.tile([128, 512], F32,
                                  tag=("sba" if sub == 0 else "sbb"),
                                  name="ps_t")
                nc.tensor.transpose(ps_t[0:128, 0:V65],
                                    oT[:, qc * 128:(qc + 1) * 128],
                                    identf[0:V65, 0:V65])
                rec = small.tile([128, 1], F32, tag="rec", name="rec")
                nc.vector.reciprocal(rec[:], ps_t[:, HD:V65])
                nc.vector.scalar_tensor_tensor(
                    out=hbuf[qc][:, HD * h:HD * (h + 1)],
                    in0=ps_t[:, 0:HD], scalar=rec[:],
                    in1=xb_s[qc][:, HD * h:HD * (h + 1)],
                    op0=ALU.mult, op1=ALU.add)

    # mm ring (created after attention closes vq); holder for closures
    mm_ring = []
    mm_state = [0]

    def mm_tile():
        t = mm_ring[mm_state[0] % len(mm_ring)]()
        mm_state[0] += 1
        return t

    def hn_block():
        """rmsnorm + transpose of hbuf for all 4 query tiles."""
        sss = small.tile([128, QTT], F32, tag="hsss", name="hsss")
        srtg = small.tile([128, QTT], F32, tag="hsrt", name="hsrt")
        invg = small.tile([128, QTT], F32, tag="hinv", name="hinv")
        for qc in range(QTT):
            scr = rot.tile([128, D], BF16, tag="hscr", name="hscr", bufs=2)
            nc.vector.scalar_tensor_tensor(
                out=scr[:], in0=hbuf[qc][:], scalar=1.0, in1=hbuf[qc][:],
                op0=ALU.mult, op1=ALU.mult, accum_out=sss[:, qc:qc + 1])
        nc.scalar.activation(srtg[:], sss[:], AF.Sqrt, bias=epsb[:],
                             scale=1.0 / D)
        nc.vector.reciprocal(invg[:], srtg[:])
        for qc in range(QTT):
            z2 = rot.tile([128, D], F32, tag="z2", name="z2", bufs=2)
            nc.vector.tensor_scalar_mul(z2[:], hbuf[qc][:],
                                        invg[:, qc:qc + 1])
            ps = mm_tile()
            for d in range(DT4):
                nc.tensor.matmul(ps[:, d * 128:(d + 1) * 128],
                                 z2[:, d * 128:(d + 1) * 128],
                                 identf[:], is_transpose=True,
                                 start=(d == 0), stop=(d == DT4 - 1))
            nc.scalar.copy(
                hnT_all[:].rearrange("p (d c) -> p d c", c=QT)[
                    :, :, qc * 128:(qc + 1) * 128],
                ps[:].rearrange("p (d c) -> p d c", c=128))

    def mlp_j(j):
        """SwiGLU hidden tile j, all 512 tokens."""
        ps2 = mm_tile()
        for dk in range(DT4):
            nc.tensor.matmul(ps2[:],
                             w1slice(w1_t, dk, j * 128, (j + 1) * 128),
                             hnT[dk][:],
                             start=(dk == 0), stop=(dk == DT4 - 1))
        su = rot.tile([128, QT], F32, tag="su", name="su", bufs=2)
        if SIM_SILU:
            a2 = rot.tile([128, QT], F32, tag="a2", name="a2")
            nc.scalar.activation(a2[:], ps2[:], AF.Identity,
                                 bias=bias_ap(B10, j))
            sg = rot.tile([128, QT], F32, tag="sg", name="sg")
            nc.scalar.activation(sg[:], ps2[:], AF.Sigmoid,
                                 bias=bias_ap(B10, j))
            nc.vector.tensor_mul(su[:], a2[:], sg[:])
        else:
            nc.scalar.activation(su[:], ps2[:], AF.Silu,
                                 bias=bias_ap(B10, j))
        ps3 = mm_tile()
        for dk in range(DT4):
            nc.tensor.matmul(ps3[:],
                             w1slice(w2_t, dk, j * 128, (j + 1) * 128),
                             hnT[dk][:],
                             start=(dk == 0), stop=(dk == DT4 - 1))
        gb = mwp2.tile([128, QT], BF16, tag=f"g{j}", name=f"g{j}")
        gbuf[j] = gb
        nc.vector.scalar_tensor_tensor(
            out=gb[:], in0=ps3[:],
            scalar=bias_ap(B20, j), in1=su[:],
            op0=ALU.add, op1=ALU.mult)

    def w3_i(i):
        """final projection output tile i + residual add."""
        ps4 = mm_tile()
        for j in range(HT):
            nc.tensor.matmul(ps4[:],
                             w3slice(j, i * 128, (i + 1) * 128),
                             gbuf[j][:],
                             start=(j == 0), stop=(j == HT - 1))
        outT = rot.tile([128, QT], F32, tag="outT", name="outT", bufs=2)
        nc.scalar.activation(outT[:], ps4[:], AF.Identity,
                             bias=bias_ap(B30, i))
        for qc in range(QTT):
            ps5 = sb_tile()
            nc.tensor.transpose(ps5[:, 0:128],
                                outT[:, qc * 128:(qc + 1) * 128],
                                identf[:])
            nc.vector.tensor_add(outbuf[qc][:, i * 128:(i + 1) * 128],
                                 ps5[:, 0:128],
                                 hbuf[qc][:, i * 128:(i + 1) * 128])
            if i == DT4 - 1:
                nc.sync.dma_start(p["out"][qc * 128:(qc + 1) * 128, :],
                                  outbuf[qc][:])

    # ---------- projections + attention: av(pr-1) woven into round pr ----
    nc.scalar.activation(warm[:], epsb[:], AF.Exp)
    nc.gpsimd.memset(
        v65_all[:].rearrange("q (t h c) -> q t h c", t=TT, c=V65)[
            :, :, :, HD:V65], 1.0)

    kq_block(0)
    for g in range(TT // 2):
        sc_g(0, g)
        v_block(g)
    for pr in range(1, DT4):
        kq_block(pr)
        av_start()
        prev = list(av_ps)
        if pr == DT4 - 1:
            # last round: weave av(2) and trail av(3) by two groups.
            # av(3) accumulates in the vq tile (both subs side by side).
            av3 = [None, None]
            for g in range(TT // 2):
                if g < 4:
                    av_ps[0], av_ps[1] = prev[0], prev[1]
                    av_chunk(pr - 1, 2 * g)
                    av_chunk(pr - 1, 2 * g + 1)
                sc_g(pr, g)
                if g == 1:
                    av3vq = vq_tile()
                    av3[0] = av3vq[:, 0:512]
                    av3[1] = av3vq[:, 512:1024]
                if g == 4:
                    av_ps[0], av_ps[1] = prev[0], prev[1]
                    av_epilogue(pr - 1)
                if g >= 2:
                    av_ps[0], av_ps[1] = av3[0], av3[1]
                    av_chunk(pr, g - 2)
            # warm the sqrt table while PE/DVE finish the tail
            nc.scalar.activation(warm[:], epsb[:], AF.Sqrt)
            av_ps[0], av_ps[1] = av3[0], av3[1]
            for g in range(TT // 2 - 2, TT // 2):
                av_chunk(pr, g)
            av_epilogue(pr)
        else:
            ring3 = sc_ring + [vq_tile]
            for g in range(TT // 2):
                av_chunk(pr - 1, g)
                sc_g(pr, g, ring3)
            av_epilogue(pr - 1)

    zpool.close()   # zT + wqkv no longer needed
    pt_pool.close()  # p tiles consumed by the last av_block
    s_scA.close()   # free sc + vq banks -> mm ring + w3 accumulators
    mm_ps = ctx.enter_context(
        tc.tile_pool(name="mm_ps", bufs=1, space="PSUM", side="right"))
    mm_ring.extend([
        lambda: mm_ps.tile([128, 512], F32, tag="mma", name="mma"),
        lambda: mm_ps.tile([128, 512], F32, tag="mmb", name="mmb"),
        lambda: mm_ps.tile([128, 512], F32, tag="mmc", name="mmc"),
        lambda: mm_ps.tile([128, 512], F32, tag="mmd", name="mmd"),
    ])
    s_mlpw2 = ExitStack()
    mwp2 = s_mlpw2.enter_context(tc.tile_pool(name="mwp2", bufs=1))
    w3_holder[0] = mwp2.tile([128, HT * D], BF16, tag="w3", name="w3")
    nc.sync.dma_start(
        w3_holder[0][:].rearrange("p (a d) -> p a d", a=HT),
        p["w3T"][:].rearrange("(a p) d -> p a d", p=128))
    hnT_all = mwp2.tile([128, DT4 * QT], BF16, tag="hnT", name="hnT")
    for d in range(DT4):
        hnT[d] = hnT_all[:, d * QT:(d + 1) * QT]

    # ---------- hn + MLP ----------
    hn_block()
    for j in range(HT):
        mlp_j(j)
    for i in range(DT4):
        w3_i(i)

    s_mlpw.close()
    s_mlpw2.close()


# ======================= host side =======================

_NC_CACHE = None


def _get_module():
    global _NC_CACHE
    if _NC_CACHE is None:
        _NC_CACHE = build_module()
    return _NC_CACHE


def host_prep(inputs):
    """Full inputs -> per-core in_maps (list of 8 dicts)."""
    f32 = np.float32
    bf16 = ml_dtypes.bfloat16
    x = np.asarray(inputs["x"], f32)
    DA = np.asarray(inputs["DA"])
    g1 = np.asarray(inputs["g1"], f32)
    g2 = np.asarray(inputs["g2"], f32)
    Wq = np.asarray(inputs["Wq"], f32)
    Wk = np.asarray(inputs["Wk"], f32)
    Wv = np.asarray(inputs["Wv"], f32)
    W1 = np.asarray(inputs["W1"], f32)
    W2 = np.asarray(inputs["W2"], f32)
    W3 = np.asarray(inputs["W3"], f32)
    bq = np.asarray(inputs["bq"], f32)
    bk = np.asarray(inputs["bk"], f32)
    bv = np.asarray(inputs["bv"], f32)
    b1 = np.asarray(inputs["b1"], f32)
    b2 = np.asarray(inputs["b2"], f32)
    b3 = np.asarray(inputs["b3"], f32)

    def wcast(a):
        return np.ascontiguousarray(a).astype(bf16)

    C = np.ascontiguousarray
    s = 1.0 / np.sqrt(HD)
    bias = np.zeros((128, NBIAS), f32)
    bias[:, BQ0:BQ0 + 4] = (bq * s).reshape(4, 128).T
    bias[:, BK0:BK0 + 4] = bk.reshape(4, 128).T
    bias[:, B10:B10 + 16] = b1.reshape(16, 128).T
    bias[:, B20:B20 + 16] = b2.reshape(16, 128).T
    bias[:, B30:B30 + 4] = b3.reshape(4, 128).T

    shared = {
        "wqT": wcast((Wq * g1[None, :]).T * s),
        "wkT": wcast((Wk * g1[None, :]).T),
        "wvT": wcast((Wv * g1[None, :]).T),
        "w1T": wcast((W1 * g2[None, :]).T),
        "w2T": wcast((W2 * g2[None, :]).T),
        "w3T": wcast(W3.T),
        "bias": bias,
    }
    maskT = [(DA[b, 0] != 0).astype(bf16).T for b in range(B)]

    in_maps = []
    for c in range(NCORES):
        b = c // (NCORES // B)
        qs = (c % (NCORES // B)) * QT
        xo = x[b, qs:qs + QT]
        in_maps.append(dict(
            shared,
            xf=C(x[b]).astype(bf16),
            xo=C(xo).astype(bf16),
            xb=C(xo + bv[None, :]).astype(bf16),
            mT=C(maskT[b][:, qs:qs + QT]),
        ))
    return in_maps


def assemble(results):
    out = np.empty((B, N, D), np.float32)
    for c in range(NCORES):
        b = c // (NCORES // B)
        qs = (c % (NCORES // B)) * QT
        out[b, qs:qs + QT] = results[c]["out"]
    return out


LAST_EXEC_NS = None


def kernel(_trace=False, **inputs):
    from concourse.bass_utils import run_bass_kernel_spmd

    global LAST_EXEC_NS
    nc = _get_module()
    in_maps = host_prep(inputs)
    res = run_bass_kernel_spmd(nc, in_maps, list(range(NCORES)), trace=_trace)
    LAST_EXEC_NS = res.exec_time_ns
    return assemble(res.results)
